# revision 3
# baseline (speedup 1.0000x reference)
"""Bass/Tile TRN2 kernel for nn_LzScaleDotAttention (B=8, L=2048, D=512).

Math per batch b (see module docstring of the nn problem):
    S[q,k]   = sum_d Q[q,d] K[k,d]
    E        = exp(S)                       # inputs are pre-scaled small, no max-sub needed
    num[k,d] = sum_q E[q,k] V[q,d]          # = E^T @ V
    den[k]   = sum_q E[q,k]
    mask[k]  = 1.0 if any(V[k,:] != 0) else 0.0
    out[k,d] = num[k,d] * mask[k]*c / (den[k]*mask[k]*c + EPS),  c = 1/sqrt(D)

The renormalisation over the query axis commutes with the E^T@V contraction
(the divisor depends only on k), so we never materialise the normalised
attention matrix: one flash-style pass over q tiles accumulates num (PSUM)
and den (SBUF f32 accumulator + one tiny cross-partition matmul with ones).

Sharding: batch dim (8) across the 8 NeuronCores, one batch per core (SPMD,
no collectives). Matmuls run in float32r (full fp32 storage, fast PE mode).
"""

import math
import os
import sys

import numpy as np

for _p in ("/opt/trn_rl_repo", "/root/.axon_site/_ro/trn_rl_repo"):
    if os.path.isdir(_p) and _p not in sys.path:
        sys.path.append(_p)

import concourse.bacc as bacc
import concourse.mybir as mybir
import concourse.tile as tile
from concourse.bass import ds, ts
from concourse.bass_utils import run_bass_kernel_spmd
from concourse.masks import make_identity

B, L, D = 8, 2048, 512
P = 128
EPS = 1e-7
N_CORES = 8

f32 = mybir.dt.float32
f32r = mybir.dt.float32r
AF = mybir.ActivationFunctionType
ALU = mybir.AluOpType


def _r(ap):
    """fp32r view: same bits as fp32, runs the PE at full rate for N>=256."""
    return ap.bitcast(f32r)


def build_program(Lb=L, Db=D, n_cores=N_CORES):
    NT = Lb // P          # 128-row tiles along q / k timesteps
    DC = Db // P          # 128-wide chunks of the feature dim
    KBW = 512             # k-block width (one PSUM bank of fp32)
    KB = Lb // KBW        # k blocks
    KT = KBW // P         # 128-wide k tiles per block
    C = 1.0 / math.sqrt(Db)

    nc = bacc.Bacc(
        "TRN2", target_bir_lowering=False, debug=False, num_devices=n_cores
    )
    q = nc.dram_tensor("q", [Lb, Db], f32, kind="ExternalInput").ap()
    k = nc.dram_tensor("k", [Lb, Db], f32, kind="ExternalInput").ap()
    v = nc.dram_tensor("v", [Lb, Db], f32, kind="ExternalInput").ap()
    out = nc.dram_tensor("out", [Lb, Db], f32, kind="ExternalOutput").ap()

    with tile.TileContext(nc) as tc:
        with (
            tc.tile_pool(name="const", bufs=1) as cpool,
            tc.tile_pool(name="qTp", bufs=NT) as qT_pool,
            tc.tile_pool(name="kTp", bufs=KB) as kT_pool,
            tc.tile_pool(name="vSp", bufs=NT) as vS_pool,
            tc.tile_pool(name="stage", bufs=4) as stage_pool,
            tc.tile_pool(name="ep", bufs=3) as e_pool,
            tc.tile_pool(name="accp", bufs=2) as acc_pool,
            tc.tile_pool(name="outp", bufs=3) as out_pool,
            tc.tile_pool(name="scp", bufs=4) as sc_pool,
            tc.tile_pool(name="ps_s", bufs=2, space="PSUM") as ps_s,
            tc.tile_pool(name="ps_num", bufs=1, space="PSUM") as ps_num,
            tc.tile_pool(name="ps_tp", bufs=2, space="PSUM") as ps_tp,
        ):
            ident = cpool.tile([P, P], f32, name="ident")
            make_identity(nc, ident)
            ones = cpool.tile([P, 1], f32, name="ones")
            nc.vector.memset(ones, 1.0)
            vmask = cpool.tile([P, NT], f32, name="vmask")

            # Persistent SBUF residents: Q^T, K^T (feature-major) and V.
            qT_t = [
                qT_pool.tile([P, DC, P], f32r, tag="qT", name=f"qT{t}")
                for t in range(NT)
            ]
            kT_b = [
                kT_pool.tile([P, DC, KBW], f32r, tag="kT", name=f"kTb{b}")
                for b in range(KB)
            ]
            vS_t = [
                vS_pool.tile([P, Db], f32r, tag="vS", name=f"vS{t}")
                for t in range(NT)
            ]

            # ---- Phase 0: load V; load + PE-transpose Q and K ----
            for t in range(NT):
                # gpsimd DMA may cast (relabel) f32 -> f32r; the PE rounds on
                # ingest anyway, so no separate rounding pass is needed
                nc.gpsimd.dma_start(vS_t[t], v[ts(t, P), :])
                nc.vector.tensor_reduce(
                    vmask[:, t : t + 1],
                    vS_t[t],
                    axis=mybir.AxisListType.X,
                    op=ALU.max,
                    apply_absolute_value=True,
                )
                kf = stage_pool.tile([P, Db], f32, tag="kf", name=f"kf{t}")
                nc.sync.dma_start(kf, k[ts(t, P), :])
                qf = stage_pool.tile([P, Db], f32, tag="qf", name=f"qf{t}")
                nc.sync.dma_start(qf, q[ts(t, P), :])
                for dc in range(DC):
                    tpk = ps_tp.tile([P, P], f32, tag="tp", name=f"tpk{t}_{dc}")
                    nc.tensor.transpose(tpk, kf[:, ts(dc, P)], ident)
                    nc.scalar.copy(kT_b[t // KT][:, dc, ts(t % KT, P)], tpk)
                    tpq = ps_tp.tile([P, P], f32, tag="tp", name=f"tpq{t}_{dc}")
                    nc.tensor.transpose(tpq, qf[:, ts(dc, P)], ident)
                    nc.vector.tensor_copy(qT_t[t][:, dc, :], tpq)
            # mask[k] = (max_d |v[k,d]|) > 0  ->  {0.0, 1.0}
            nc.vector.tensor_scalar(vmask, vmask, 0.0, None, op0=ALU.is_gt)

            # ---- Main flash loop over k blocks ----
            for kb in range(KB):
                nums = [
                    ps_num.tile([P, Db], f32, tag=f"num{kt}", name=f"num{kb}_{kt}")
                    for kt in range(KT)
                ]
                acc = acc_pool.tile([P, KBW], f32, tag="acc", name=f"acc{kb}")
                e_tiles = {}
                # software pipeline: stage-1 (scores+exp) runs one q-tile
                # ahead of stage-2 (E^T @ V) so the PE never waits on ACT
                for qt in range(NT + 1):
                    if qt < NT:
                        s_ps = ps_s.tile([P, KBW], f32, tag="s", name=f"s{kb}_{qt}")
                        for dc in range(DC):
                            nc.tensor.matmul(
                                s_ps,
                                qT_t[qt][:, dc, :],
                                kT_b[kb][:, dc, :],
                                start=(dc == 0),
                                stop=(dc == DC - 1),
                            )
                        e = e_pool.tile([P, KBW], f32r, tag="e", name=f"e{kb}_{qt}")
                        nc.scalar.activation(e, s_ps, AF.Exp)
                        if qt == 0:
                            nc.vector.tensor_copy(acc, e)
                        else:
                            nc.vector.tensor_add(acc, acc, e)
                        e_tiles[qt] = e
                    if qt >= 1:
                        ep = e_tiles.pop(qt - 1)
                        for kt in range(KT):
                            nc.tensor.matmul(
                                nums[kt],
                                ep[:, ts(kt, P)],
                                vS_t[qt - 1],
                                start=(qt - 1 == 0),
                                stop=(qt - 1 == NT - 1),
                            )
                # close the k block: den, scale, writeback
                for kt in range(KT):
                    j = kb * KT + kt
                    dps = ps_tp.tile([P, 1], f32, tag="tp", name=f"dps{j}")
                    nc.tensor.matmul(
                        dps, acc[:, ts(kt, P)], ones, start=True, stop=True
                    )
                    scl = sc_pool.tile([P, 1], f32, tag="scl", name=f"scl{j}")
                    nc.vector.tensor_mul(scl, dps, vmask[:, j : j + 1])
                    nc.vector.tensor_scalar(
                        scl, scl, C, EPS, op0=ALU.mult, op1=ALU.add
                    )
                    rcp = sc_pool.tile([P, 1], f32, tag="rcp", name=f"rcp{j}")
                    nc.vector.reciprocal(rcp, scl)
                    nc.vector.tensor_scalar(rcp, rcp, C, None, op0=ALU.mult)
                    nc.vector.tensor_mul(rcp, rcp, vmask[:, j : j + 1])
                    o = out_pool.tile([P, Db], f32, tag="o", name=f"o{j}")
                    nc.scalar.mul(o, nums[kt], rcp)
                    nc.sync.dma_start(out[ts(j, P), :], o)

    return nc


_cache = {}


def _get_compiled(Lb=L, Db=D):
    key = (Lb, Db)
    if key not in _cache:
        nc = build_program(Lb, Db)
        nc.compile()
        _cache[key] = nc
    return _cache[key]


def run(q, k, v, trace=False):
    nc = _get_compiled()
    q = np.ascontiguousarray(q, dtype=np.float32)
    k = np.ascontiguousarray(k, dtype=np.float32)
    v = np.ascontiguousarray(v, dtype=np.float32)
    in_maps = [
        {"q": q[i], "k": k[i], "v": v[i]} for i in range(N_CORES)
    ]
    res = run_bass_kernel_spmd(nc, in_maps, list(range(N_CORES)), trace=trace)
    out = np.stack([res.results[i]["out"] for i in range(N_CORES)], axis=0)
    return out.astype(np.float32, copy=False), res


def kernel(q, k, v):
    out, _ = run(q, k, v, trace=False)
    return out
